# revision 17
# baseline (speedup 1.0000x reference)
"""Trainium2 Bass kernel for nn_MaxTimesPlusOpeningLiftingP4.

Computation (per rotation i of 4):
  ero[u,c,f]  = min_p (x[u+d_p, c] - ke_i[p,c,f]) * inva_i[p,c,f]
  res[u,f]    = sum_c max_p (tk_i[p,c,f] * ero_pad[u+d_p, c, f] + k_i[p,c,f])
with SAME zero padding on both x and ero, 5x5 window (P=25).

Device layout: 120 SBUF partitions = (c=3, f=8, j=5 row-chunks); pixels on
the free dim in padded per-chunk buffers of 30 rows x 132 cols, with the
core's TWO images interleaved element-wise (w-major, b-inner) so both are
processed by every instruction (halves instruction-count overheads) and
every 5x5 shift stays a flat AP offset.  Per rotation:
  stage 1: p=0 via DVE tensor_scalar(mult,add); p>=1: affine on ACT
           (Identity, per-partition scale/bias APs) or on DVE
           (tensor_scalar, 4x in fp16) for DVE_AFFINE of the 24 terms,
           then DVE tensor_tensor(min) accumulate (2x in fp16).
  halo exchange + zero-fixups via SBUF-SBUF DMAs.
  stage 2: same with (tk, k) and max.
  c-sum via TensorE matmul with a 0/1 selection matrix -> PSUM,
  ACT copy -> SBUF, DMA out.
Sharding: pure data parallel, 2 images per core on 8 cores.
Host does weight rotation prep, x replication/interleave, and output
reassembly (host work is not on the device clock).
"""
import numpy as np

EPS = 1e-7
B, H, W, C = 16, 128, 128, 3
KH, KW, F = 5, 5, 8
P = KH * KW
NJ = 5
ROWS = [26, 26, 26, 26, 24]
CH_START = [0, 26, 52, 78, 104]
RB = 30
WP = 132
NPART = 120           # (c,f,j): partition = (c*8+f)*5 + j
NCORES = 8
BPC = B // NCORES     # images per core (interleaved in the free dim)
IL = BPC              # interleave factor
WPB = WP * IL         # padded row in elements (264)
FDB = 26 * 128 * IL   # free size per op (6656); j=4 rows 24,25 are garbage
USE_FP16 = True       # fp16 tensors: DVE TT 2x, TS 4x; rel err ~2e-3
DVE_AFFINE = 7        # of the 24 p>=1 affine terms per stage, how many run
                      # on DVE tensor_scalar instead of ACT (engine balance)

_CACHE = {}


def _part(c, f, j):
    return (c * 8 + f) * 5 + j


def _make_weights(kernel, timesKernel):
    """[120, 400] f32; col = (rot*25+p)*4 + slot, slot 0=a 1=b 2=tk 3=k."""
    kernel = np.asarray(kernel, np.float32)
    timesKernel = np.asarray(timesKernel, np.float32)
    k_ero = kernel[::-1, ::-1]
    t_ero = timesKernel[::-1, ::-1]
    Wt = np.zeros((NPART, 4 * P * 4), np.float32)
    for i in range(4):
        k_rot = np.rot90(kernel, k=i, axes=(0, 1)).reshape(P, C, F)
        tk_rot = np.rot90(timesKernel, k=i, axes=(0, 1)).reshape(P, C, F)
        ke_rot = np.rot90(k_ero, k=i, axes=(0, 1)).reshape(P, C, F)
        tke_rot = np.rot90(t_ero, k=i, axes=(0, 1)).reshape(P, C, F)
        a = (1.0 / (tke_rot.astype(np.float64) + EPS)).astype(np.float32)
        b = (-ke_rot * a).astype(np.float32)
        for c in range(C):
            for f in range(F):
                pi = (c * 8 + f) * 5 + np.arange(NJ)
                for p in range(P):
                    col = (i * P + p) * 4
                    Wt[pi, col + 0] = a[p, c, f]
                    Wt[pi, col + 1] = b[p, c, f]
                    Wt[pi, col + 2] = tk_rot[p, c, f]
                    Wt[pi, col + 3] = k_rot[p, c, f]
    return Wt


def _make_csum():
    S = np.zeros((NPART, 40), np.float32)
    for c in range(C):
        for f in range(F):
            for j in range(NJ):
                S[_part(c, f, j), f * NJ + j] = 1.0
    return S


def _build_xrep(x):
    """x [BPC,H,W,C] -> [120, RB*WPB] padded chunks, b interleaved inner."""
    xpad = np.zeros((BPC, H + 6, W + 4, C), np.float32)
    xpad[:, 2:2 + H, 2:2 + W, :] = x
    xr = np.zeros((NPART, RB, WP, IL), np.float32)
    for c in range(C):
        for j in range(NJ):
            blk = xpad[:, CH_START[j]:CH_START[j] + RB, :, c]  # [IL,RB,WP]
            blk = np.moveaxis(blk, 0, -1)                      # [RB,WP,IL]
            for f in range(F):
                xr[(c * 8 + f) * 5 + j] = blk
    return xr.reshape(NPART, RB * WPB)


def _build_program():
    import concourse.bass as bass
    import concourse.bacc as bacc
    import concourse.mybir as mybir
    import concourse.tile as tile

    f32 = mybir.dt.float32
    dt_c = mybir.dt.float16 if USE_FP16 else mybir.dt.float32
    Alu = mybir.AluOpType
    Act = mybir.ActivationFunctionType

    nc = bacc.Bacc("TRN2", target_bir_lowering=False, debug=False,
                   num_devices=NCORES)
    x_in = nc.dram_tensor("x8", [NPART, RB * WPB], dt_c, kind="ExternalInput")
    wts_in = nc.dram_tensor("wts", [NPART, 4 * P * 4], f32, kind="ExternalInput")
    cs_in = nc.dram_tensor("csum", [NPART, 40], dt_c, kind="ExternalInput")
    out_d = nc.dram_tensor("out", [4, 40, FDB], dt_c, kind="ExternalOutput")

    def sb_ap(t, part_off, free_off, dims):
        row = t.shape[1]
        ap = [[sp * row, cnt] for sp, cnt in dims[0]]
        ap += [[se, cnt] for se, cnt in dims[1]]
        return bass.AP(tensor=t.tensor,
                       offset=t.offset + part_off * row + free_off, ap=ap)

    NCHUNK = 4            # c-sum PSUM chunking
    CHW = FDB // NCHUNK   # 1664 elems -> 4-bank PSUM tiles

    with tile.TileContext(nc) as tc:
        with (
            tc.tile_pool(name="singles", bufs=1) as singles,
            tc.tile_pool(name="tmp", bufs=8) as tmp_pool,
            tc.tile_pool(name="acc2", bufs=2) as acc2_pool,
            tc.tile_pool(name="res", bufs=2) as res_pool,
            tc.tile_pool(name="psum", bufs=2, space="PSUM") as psum_pool,
        ):
            wts = singles.tile([NPART, 4 * P * 4], f32, tag="wts")
            cs = singles.tile([NPART, 40], dt_c, tag="cs")
            zeros = singles.tile([NPART, 2 * WPB], dt_c, tag="zeros")
            nc.sync.dma_start(out=wts[:], in_=wts_in[:])
            nc.sync.dma_start(out=cs[:], in_=cs_in[:])
            nc.vector.memset(zeros[:], 0.0)

            xr = singles.tile([NPART, RB * WPB], dt_c, tag="xrep", name="xrep")
            eros = [singles.tile([NPART, RB * WPB], dt_c, tag=f"ero{k}",
                                 name=f"ero{k}") for k in range(2)]
            for t in eros:
                nc.vector.memset(t[:], 0.0)
            nc.sync.dma_start(out=xr[:], in_=x_in[:])

            def wcol(rot, p, slot):
                return (rot * P + p) * 4 + slot

            def shift_view(t, p):
                return sb_ap(t, 0, (p // 5) * WPB + (p % 5) * IL,
                             [[(1, NPART)], [(WPB, 26), (1, 128 * IL)]])

            def interior(t):
                return sb_ap(t, 0, 2 * WPB + 2 * IL,
                             [[(1, NPART)], [(WPB, 26), (1, 128 * IL)]])

            # stage-2 starts with dh=0 terms so its first ops depend only
            # on the garbage-zero DMA, not the halo DMAs
            ORDER2 = ([12, 10, 11, 13, 14] + list(range(5, 10))
                      + list(range(15, 20)) + list(range(0, 5))
                      + list(range(20, 25)))

            def stage(rot, src, dst_ap, s_mul, s_add, accop, order=None):
                """25-term affine + accumulate into dst_ap."""
                order = order or list(range(P))
                nc.vector.tensor_scalar(
                    out=dst_ap, in0=shift_view(src, order[0]),
                    scalar1=s_mul(order[0]), scalar2=s_add(order[0]),
                    op0=Alu.mult, op1=Alu.add)
                for i_p, p in enumerate(order[1:]):
                    t = tmp_pool.tile([NPART, FDB], dt_c, tag="tmp", name="tmp")
                    if DVE_AFFINE and (i_p * DVE_AFFINE) % 24 < DVE_AFFINE:
                        nc.vector.tensor_scalar(
                            out=t[:], in0=shift_view(src, p),
                            scalar1=s_mul(p), scalar2=s_add(p),
                            op0=Alu.mult, op1=Alu.add)
                    else:
                        nc.scalar.activation(
                            out=t[:], in_=shift_view(src, p),
                            func=Act.Identity, bias=s_add(p), scale=s_mul(p))
                    nc.vector.tensor_tensor(out=dst_ap, in0=t[:], in1=dst_ap,
                                            op=accop)

            for rot in range(4):
                ero = eros[rot % 2]
                # ---------------- stage 1: erosion ----------------
                stage(rot, xr, interior(ero),
                      lambda p: wts[:, wcol(rot, p, 0):wcol(rot, p, 0) + 1],
                      lambda p: wts[:, wcol(rot, p, 1):wcol(rot, p, 1) + 1],
                      Alu.min)
                # zero j=4 garbage rows (buffer rows 26,27)
                nc.sync.dma_start(
                    out=sb_ap(ero, 4, 26 * WPB, [[(5, 24)], [(1, 2 * WPB)]]),
                    in_=sb_ap(zeros, 4, 0, [[(5, 24)], [(1, 2 * WPB)]]))
                # halo exchange: top rows 0,1 of j <- j-1 rows 26,27
                for jj in range(1, NJ):
                    nc.sync.dma_start(
                        out=sb_ap(ero, jj, 0, [[(5, 24)], [(1, 2 * WPB)]]),
                        in_=sb_ap(ero, jj - 1, 26 * WPB,
                                  [[(5, 24)], [(1, 2 * WPB)]]))
                # bottom rows 28,29 of j <- j+1 rows 2,3
                for jj in range(NJ - 1):
                    nc.sync.dma_start(
                        out=sb_ap(ero, jj, 28 * WPB, [[(5, 24)], [(1, 2 * WPB)]]),
                        in_=sb_ap(ero, jj + 1, 2 * WPB,
                                  [[(5, 24)], [(1, 2 * WPB)]]))
                # ---------------- stage 2: dilation ----------------
                acc2 = acc2_pool.tile([NPART, FDB], dt_c, tag="acc2",
                                      name="acc2")
                stage(rot, ero, acc2[:],
                      lambda p: wts[:, wcol(rot, p, 2):wcol(rot, p, 2) + 1],
                      lambda p: wts[:, wcol(rot, p, 3):wcol(rot, p, 3) + 1],
                      Alu.max, order=ORDER2)
                # ---------------- c-sum + writeout ----------------
                res = res_pool.tile([40, FDB], dt_c, tag="res", name="res")
                for h in range(NCHUNK):
                    ps = psum_pool.tile([40, CHW], f32, tag="ps", name="ps")
                    base = h * CHW
                    for k in range((CHW + 511) // 512):
                        n0 = k * 512
                        n1 = min(CHW, n0 + 512)
                        nc.tensor.matmul(ps[:, n0:n1], cs[:, 0:40],
                                         acc2[:, base + n0:base + n1],
                                         start=True, stop=True)
                    nc.scalar.copy(res[:, base:base + CHW], ps[:])
                nc.sync.dma_start(out=out_d[rot], in_=res[:])
    nc.compile()
    return nc


def _get_program():
    if "nc" not in _CACHE:
        _CACHE["nc"] = _build_program()
    return _CACHE["nc"]


def kernel(x, kernel, timesKernel):
    x = np.ascontiguousarray(np.asarray(x, np.float32))
    Wt = _make_weights(kernel, timesKernel)
    S = _make_csum()

    nc = _get_program()
    from concourse.bass_utils import run_bass_kernel_spmd
    dt_np = np.float16 if USE_FP16 else np.float32
    in_maps = []
    for i in range(NCORES):
        xrh = _build_xrep(x[i * BPC:(i + 1) * BPC]).astype(dt_np)
        in_maps.append({"x8": xrh, "wts": Wt, "csum": S.astype(dt_np)})

    import os
    trace = os.environ.get("BASS_TRACE", "0") == "1"
    r = run_bass_kernel_spmd(nc, in_maps, core_ids=list(range(NCORES)),
                             trace=trace)
    _CACHE["last_results"] = r
    outs = [m["out"] for m in r.results]

    full = np.empty((B, 4, H, W, F), np.float32)
    for i in range(NCORES):
        O = outs[i].astype(np.float32).reshape(4, 40, 26, 128, IL)
        for rot in range(4):
            for f in range(F):
                for j in range(NJ):
                    rws = ROWS[j]
                    for bb in range(BPC):
                        full[i * BPC + bb, rot,
                             CH_START[j]:CH_START[j] + rws, :, f] = \
                            O[rot, f * NJ + j, :rws, :, bb]
    return full


# revision 19
# speedup vs baseline: 1.0094x; 1.0094x over previous
"""Trainium2 Bass kernel for nn_MaxTimesPlusOpeningLiftingP4.

Computation (per rotation i of 4):
  ero[u,c,f]  = min_p (x[u+d_p, c] - ke_i[p,c,f]) * inva_i[p,c,f]
  res[u,f]    = sum_c max_p (tk_i[p,c,f] * ero_pad[u+d_p, c, f] + k_i[p,c,f])
with SAME zero padding on both x and ero, 5x5 window (P=25).

Device layout: 120 SBUF partitions = (c=3, f=8, j=5 row-chunks); pixels on
the free dim in padded per-chunk buffers of 30 rows x 132 cols, with the
core's TWO images interleaved element-wise (w-major, b-inner) so both are
processed by every instruction (halves instruction-count overheads) and
every 5x5 shift stays a flat AP offset.  Per rotation:
  stage 1: p=0 via DVE tensor_scalar(mult,add); p>=1: affine on ACT
           (Identity, per-partition scale/bias APs) or on DVE
           (tensor_scalar, 4x in fp16) for DVE_AFFINE of the 24 terms,
           then DVE tensor_tensor(min) accumulate (2x in fp16).
  halo exchange + zero-fixups via SBUF-SBUF DMAs.
  stage 2: same with (tk, k) and max.
  c-sum via TensorE matmul with a 0/1 selection matrix -> PSUM,
  ACT copy -> SBUF, DMA out.
Sharding: pure data parallel, 2 images per core on 8 cores.
Host does weight rotation prep, x replication/interleave, and output
reassembly (host work is not on the device clock).
"""
import numpy as np

EPS = 1e-7
B, H, W, C = 16, 128, 128, 3
KH, KW, F = 5, 5, 8
P = KH * KW
NJ = 5
ROWS = [26, 26, 26, 26, 24]
CH_START = [0, 26, 52, 78, 104]
RB = 30
WP = 132
NPART = 120           # (c,f,j): partition = (c*8+f)*5 + j
NCORES = 8
BPC = B // NCORES     # images per core (interleaved in the free dim)
IL = BPC              # interleave factor
WPB = WP * IL         # padded row in elements (264)
FDB = 26 * 128 * IL   # free size per op (6656); j=4 rows 24,25 are garbage
USE_FP16 = True       # fp16 tensors: DVE TT 2x, TS 4x; rel err ~2e-3
DVE_AFFINE = 7        # of the 24 p>=1 affine terms per stage, how many run
                      # on DVE tensor_scalar instead of ACT (engine balance)

_CACHE = {}


def _part(c, f, j):
    return (c * 8 + f) * 5 + j


def _make_weights(kernel, timesKernel):
    """[120, 400] f32; col = (rot*25+p)*4 + slot, slot 0=a 1=b 2=tk 3=k."""
    kernel = np.asarray(kernel, np.float32)
    timesKernel = np.asarray(timesKernel, np.float32)
    k_ero = kernel[::-1, ::-1]
    t_ero = timesKernel[::-1, ::-1]
    Wt = np.zeros((NPART, 4 * P * 4), np.float32)
    for i in range(4):
        k_rot = np.rot90(kernel, k=i, axes=(0, 1)).reshape(P, C, F)
        tk_rot = np.rot90(timesKernel, k=i, axes=(0, 1)).reshape(P, C, F)
        ke_rot = np.rot90(k_ero, k=i, axes=(0, 1)).reshape(P, C, F)
        tke_rot = np.rot90(t_ero, k=i, axes=(0, 1)).reshape(P, C, F)
        a = (1.0 / (tke_rot.astype(np.float64) + EPS)).astype(np.float32)
        b = (-ke_rot * a).astype(np.float32)
        for c in range(C):
            for f in range(F):
                pi = (c * 8 + f) * 5 + np.arange(NJ)
                for p in range(P):
                    col = (i * P + p) * 4
                    Wt[pi, col + 0] = a[p, c, f]
                    Wt[pi, col + 1] = b[p, c, f]
                    Wt[pi, col + 2] = tk_rot[p, c, f]
                    Wt[pi, col + 3] = k_rot[p, c, f]
    return Wt


def _make_csum():
    S = np.zeros((NPART, 40), np.float32)
    for c in range(C):
        for f in range(F):
            for j in range(NJ):
                S[_part(c, f, j), f * NJ + j] = 1.0
    return S


def _build_xrep(x):
    """x [BPC,H,W,C] -> [120, RB*WPB] padded chunks, b interleaved inner."""
    xpad = np.zeros((BPC, H + 6, W + 4, C), np.float32)
    xpad[:, 2:2 + H, 2:2 + W, :] = x
    xr = np.zeros((NPART, RB, WP, IL), np.float32)
    for c in range(C):
        for j in range(NJ):
            blk = xpad[:, CH_START[j]:CH_START[j] + RB, :, c]  # [IL,RB,WP]
            blk = np.moveaxis(blk, 0, -1)                      # [RB,WP,IL]
            for f in range(F):
                xr[(c * 8 + f) * 5 + j] = blk
    return xr.reshape(NPART, RB * WPB)


def _build_program():
    import concourse.bass as bass
    import concourse.bacc as bacc
    import concourse.mybir as mybir
    import concourse.tile as tile

    f32 = mybir.dt.float32
    dt_c = mybir.dt.float16 if USE_FP16 else mybir.dt.float32
    Alu = mybir.AluOpType
    Act = mybir.ActivationFunctionType

    nc = bacc.Bacc("TRN2", target_bir_lowering=False, debug=False,
                   num_devices=NCORES)
    x_in = nc.dram_tensor("x8", [NPART, RB * WPB], dt_c, kind="ExternalInput")
    wts_in = nc.dram_tensor("wts", [NPART, 4 * P * 4], f32, kind="ExternalInput")
    cs_in = nc.dram_tensor("csum", [NPART, 40], dt_c, kind="ExternalInput")
    out_d = nc.dram_tensor("out", [4, 40, FDB], dt_c, kind="ExternalOutput")

    def sb_ap(t, part_off, free_off, dims):
        row = t.shape[1]
        ap = [[sp * row, cnt] for sp, cnt in dims[0]]
        ap += [[se, cnt] for se, cnt in dims[1]]
        return bass.AP(tensor=t.tensor,
                       offset=t.offset + part_off * row + free_off, ap=ap)

    NCHUNK = 4            # c-sum PSUM chunking
    CHW = FDB // NCHUNK   # 1664 elems -> 4-bank PSUM tiles

    with tile.TileContext(nc) as tc:
        with (
            tc.tile_pool(name="singles", bufs=1) as singles,
            tc.tile_pool(name="tmp", bufs=6) as tmp_pool,
            tc.tile_pool(name="acc2", bufs=3) as acc2_pool,
            tc.tile_pool(name="res", bufs=3) as res_pool,
            tc.tile_pool(name="psum", bufs=2, space="PSUM") as psum_pool,
        ):
            wts = singles.tile([NPART, 4 * P * 4], f32, tag="wts")
            cs = singles.tile([NPART, 40], dt_c, tag="cs")
            zeros = singles.tile([NPART, 2 * WPB], dt_c, tag="zeros")
            nc.sync.dma_start(out=wts[:], in_=wts_in[:])
            nc.sync.dma_start(out=cs[:], in_=cs_in[:])
            nc.vector.memset(zeros[:], 0.0)

            xr = singles.tile([NPART, RB * WPB], dt_c, tag="xrep", name="xrep")
            eros = [singles.tile([NPART, RB * WPB], dt_c, tag=f"ero{k}",
                                 name=f"ero{k}") for k in range(2)]
            for t in eros:
                nc.vector.memset(t[:], 0.0)
            nc.sync.dma_start(out=xr[:], in_=x_in[:])

            def wcol(rot, p, slot):
                return (rot * P + p) * 4 + slot

            def shift_view(t, p):
                return sb_ap(t, 0, (p // 5) * WPB + (p % 5) * IL,
                             [[(1, NPART)], [(WPB, 26), (1, 128 * IL)]])

            def interior(t):
                return sb_ap(t, 0, 2 * WPB + 2 * IL,
                             [[(1, NPART)], [(WPB, 26), (1, 128 * IL)]])

            # stage-2 starts with dh=0 terms so its first ops depend only
            # on the garbage-zero DMA, not the halo DMAs
            ORDER2 = ([12, 10, 11, 13, 14] + list(range(5, 10))
                      + list(range(15, 20)) + list(range(0, 5))
                      + list(range(20, 25)))

            def stage(rot, src, dst_ap, s_mul, s_add, accop, order=None):
                """25-term affine + accumulate into dst_ap."""
                order = order or list(range(P))
                nc.vector.tensor_scalar(
                    out=dst_ap, in0=shift_view(src, order[0]),
                    scalar1=s_mul(order[0]), scalar2=s_add(order[0]),
                    op0=Alu.mult, op1=Alu.add)
                for i_p, p in enumerate(order[1:]):
                    t = tmp_pool.tile([NPART, FDB], dt_c, tag="tmp", name="tmp")
                    if DVE_AFFINE and (i_p * DVE_AFFINE) % 24 < DVE_AFFINE:
                        nc.vector.tensor_scalar(
                            out=t[:], in0=shift_view(src, p),
                            scalar1=s_mul(p), scalar2=s_add(p),
                            op0=Alu.mult, op1=Alu.add)
                    else:
                        nc.scalar.activation(
                            out=t[:], in_=shift_view(src, p),
                            func=Act.Identity, bias=s_add(p), scale=s_mul(p))
                    nc.vector.tensor_tensor(out=dst_ap, in0=t[:], in1=dst_ap,
                                            op=accop)

            def csum_out(rot, acc2):
                res = res_pool.tile([40, FDB], dt_c, tag="res", name="res")
                for h in range(NCHUNK):
                    ps = psum_pool.tile([40, CHW], f32, tag="ps", name="ps")
                    base = h * CHW
                    for k in range((CHW + 511) // 512):
                        n0 = k * 512
                        n1 = min(CHW, n0 + 512)
                        nc.tensor.matmul(ps[:, n0:n1], cs[:, 0:40],
                                         acc2[:, base + n0:base + n1],
                                         start=True, stop=True)
                    nc.scalar.copy(res[:, base:base + CHW], ps[:])
                nc.sync.dma_start(out=out_d[rot], in_=res[:])

            pending = None
            for rot in range(4):
                ero = eros[rot % 2]
                # ---------------- stage 1: erosion ----------------
                stage(rot, xr, interior(ero),
                      lambda p: wts[:, wcol(rot, p, 0):wcol(rot, p, 0) + 1],
                      lambda p: wts[:, wcol(rot, p, 1):wcol(rot, p, 1) + 1],
                      Alu.min)
                # previous rotation's c-sum, deferred so ACT's stream is not
                # blocked on it at the rotation boundary
                if pending is not None:
                    csum_out(*pending)
                # zero j=4 garbage rows (buffer rows 26,27)
                nc.sync.dma_start(
                    out=sb_ap(ero, 4, 26 * WPB, [[(5, 24)], [(1, 2 * WPB)]]),
                    in_=sb_ap(zeros, 4, 0, [[(5, 24)], [(1, 2 * WPB)]]))
                # halo exchange: top rows 0,1 of j <- j-1 rows 26,27
                for jj in range(1, NJ):
                    nc.sync.dma_start(
                        out=sb_ap(ero, jj, 0, [[(5, 24)], [(1, 2 * WPB)]]),
                        in_=sb_ap(ero, jj - 1, 26 * WPB,
                                  [[(5, 24)], [(1, 2 * WPB)]]))
                # bottom rows 28,29 of j <- j+1 rows 2,3
                for jj in range(NJ - 1):
                    nc.sync.dma_start(
                        out=sb_ap(ero, jj, 28 * WPB, [[(5, 24)], [(1, 2 * WPB)]]),
                        in_=sb_ap(ero, jj + 1, 2 * WPB,
                                  [[(5, 24)], [(1, 2 * WPB)]]))
                # ---------------- stage 2: dilation ----------------
                acc2 = acc2_pool.tile([NPART, FDB], dt_c, tag="acc2",
                                      name="acc2")
                stage(rot, ero, acc2[:],
                      lambda p: wts[:, wcol(rot, p, 2):wcol(rot, p, 2) + 1],
                      lambda p: wts[:, wcol(rot, p, 3):wcol(rot, p, 3) + 1],
                      Alu.max, order=ORDER2)
                pending = (rot, acc2)
            csum_out(*pending)
    nc.compile()
    return nc


def _get_program():
    if "nc" not in _CACHE:
        _CACHE["nc"] = _build_program()
    return _CACHE["nc"]


def kernel(x, kernel, timesKernel):
    x = np.ascontiguousarray(np.asarray(x, np.float32))
    Wt = _make_weights(kernel, timesKernel)
    S = _make_csum()

    nc = _get_program()
    from concourse.bass_utils import run_bass_kernel_spmd
    dt_np = np.float16 if USE_FP16 else np.float32
    in_maps = []
    for i in range(NCORES):
        xrh = _build_xrep(x[i * BPC:(i + 1) * BPC]).astype(dt_np)
        in_maps.append({"x8": xrh, "wts": Wt, "csum": S.astype(dt_np)})

    import os
    trace = os.environ.get("BASS_TRACE", "0") == "1"
    r = run_bass_kernel_spmd(nc, in_maps, core_ids=list(range(NCORES)),
                             trace=trace)
    _CACHE["last_results"] = r
    outs = [m["out"] for m in r.results]

    full = np.empty((B, 4, H, W, F), np.float32)
    for i in range(NCORES):
        O = outs[i].astype(np.float32).reshape(4, 40, 26, 128, IL)
        for rot in range(4):
            for f in range(F):
                for j in range(NJ):
                    rws = ROWS[j]
                    for bb in range(BPC):
                        full[i * BPC + bb, rot,
                             CH_START[j]:CH_START[j] + rws, :, f] = \
                            O[rot, f * NJ + j, :rws, :, bb]
    return full
